# revision 1
# baseline (speedup 1.0000x reference)
"""Trainium2 Bass kernel for MinGPT forward (nn_MinGPT_58557584113675).

Data-parallel over batch (64) across 8 NeuronCores; weights replicated.

Per-core design (feature-major activations [C=128 partitions, N=2048 tokens]):
- LayerNorm: token sums via col-tiled all-ones matmuls (replicated 32-row
  strips, no garbage rows), small finalize, per-token mu/rstd broadcast back
  to [128, N] with K=1 ones-row matmuls, two-pass fused apply.
- Attention: heads live in 32-partition strips (4 per group, zero-padded) so
  per-head [16, *] matmul operands sit at legal 32-aligned partition bases and
  the four heads of a group run as concurrent row-tiles.  S^T = k^T-slice @ q
  (keys on partitions); exp on ScalarE without max subtraction (|score|<0.15);
  causal mask = one batched DVE multiply per batch; AV = col-tiled M=32
  matmuls with lhsT = [v_h | ones | 0-pad] so softmax denominators land in
  PSUM row 32j+16; denominators leave via strided SBUF DMA, bulk reciprocal,
  PE broadcast, and a fused evacuation multiply.
- FFN fp16, relu fused into PSUM evacuation (ACT/DVE alternating).
- Final LN + LM head fp32r, token-major (x-chunk stationary) so logits DMA
  out as contiguous rows.

Residual stream fp32r; attention/FFN internals fp16.
"""

import sys

import numpy as np

sys.path.insert(0, "/opt/trn_rl_repo")

from contextlib import ExitStack

import concourse.bass as bass
import concourse.tile as tile
from concourse import bacc, mybir
from concourse.bass_utils import run_bass_kernel_spmd

_PINNED_SET = "natural_log_exp_and_others"


def _install_act_table_pin():
    import functools
    import concourse.hw_specs as hw_specs

    if getattr(bacc, "_act_tables_pinned", False):
        return
    orig_fn = hw_specs.get_activation_tables
    pinned = {
        mybir.ActivationFunctionType.Exp,
        mybir.ActivationFunctionType.Ln,
        mybir.ActivationFunctionType.Copy,
        mybir.ActivationFunctionType.Identity,
        mybir.ActivationFunctionType.Relu,
    }

    @functools.cache
    def wrapper(module_arch):
        tables = dict(orig_fn(module_arch))
        assert pinned <= tables[_PINNED_SET]
        return {
            name: (funcs if name == _PINNED_SET else funcs - pinned)
            for name, funcs in tables.items()
        }

    bacc.get_activation_tables = wrapper
    bacc._act_tables_pinned = True

F32 = mybir.dt.float32
F32R = mybir.dt.float32r
F16 = mybir.dt.float16
U32 = mybir.dt.uint32
AF = mybir.ActivationFunctionType
OP = mybir.AluOpType

VOCAB = 3149
T = 256
C = 128
H = 8
D = 16
NLAYER = 6
FF = 512
EPS = 1e-5
NCORES = 8
BPC = 64 // NCORES
N = BPC * T                  # 2048 tokens/core
TCH = N // 128               # 16
SCH = N // 512               # 4
VCH = (VOCAB + 511) // 512   # 7
VPAD = VCH * 512


def _strip_pack_cols(w_hd):
    """[C, H*D] -> two [C, 128] with head j of each half at cols 32j..32j+16,
    zero padding elsewhere."""
    A = np.zeros((C, 128), w_hd.dtype)
    B = np.zeros((C, 128), w_hd.dtype)
    for j in range(4):
        A[:, 32 * j : 32 * j + 16] = w_hd[:, 16 * j : 16 * j + 16]
        B[:, 32 * j : 32 * j + 16] = w_hd[:, 16 * (4 + j) : 16 * (4 + j) + 16]
    return A, B


def _strip_pack_rows(vec_hd):
    """[H*D] -> two [128, 1] striped bias columns."""
    A = np.zeros((128, 1), np.float32)
    B = np.zeros((128, 1), np.float32)
    for j in range(4):
        A[32 * j : 32 * j + 16, 0] = vec_hd[16 * j : 16 * j + 16]
        B[32 * j : 32 * j + 16, 0] = vec_hd[16 * (4 + j) : 16 * (4 + j) + 16]
    return A, B


def host_prep(inputs):
    f32, f16 = np.float32, np.float16
    g = {}
    tok_emb = np.asarray(inputs["tok_emb"], f32)
    pos_emb = np.asarray(inputs["pos_emb"], f32)
    g["tok_emb"] = tok_emb
    g["posT"] = np.ascontiguousarray(pos_emb[:T].T).astype(f32)

    scale = np.float32(C ** -0.5)
    ln1_g = np.asarray(inputs["ln1_g"], f32); ln1_b = np.asarray(inputs["ln1_b"], f32)
    ln2_g = np.asarray(inputs["ln2_g"], f32); ln2_b = np.asarray(inputs["ln2_b"], f32)
    Wq = np.asarray(inputs["Wq"], f32); Wk = np.asarray(inputs["Wk"], f32)
    Wv = np.asarray(inputs["Wv"], f32); Wproj = np.asarray(inputs["Wproj"], f32)
    bproj = np.asarray(inputs["bproj"], f32)
    W1 = np.asarray(inputs["W1"], f32); b1 = np.asarray(inputs["b1"], f32)
    W2 = np.asarray(inputs["W2"], f32); b2 = np.asarray(inputs["b2"], f32)
    Lw = Wq.shape[0]

    WqTA = np.zeros((Lw, C, 128), f16); WqTB = np.zeros((Lw, C, 128), f16)
    WkTA = np.zeros((Lw, C, 128), f16); WkTB = np.zeros((Lw, C, 128), f16)
    WvT = np.zeros((Lw, C, C), f16)
    bqA = np.zeros((Lw, 128, 1), f32); bqB = np.zeros((Lw, 128, 1), f32)
    bkA = np.zeros((Lw, 128, 1), f32); bkB = np.zeros((Lw, 128, 1), f32)
    bv_tile = np.zeros((Lw, 128, C), f16)
    for l in range(Lw):
        wq = Wq[l].transpose(1, 0, 2).reshape(C, C)   # [c, (h d)]
        wk = Wk[l].transpose(1, 0, 2).reshape(C, C)
        wv = Wv[l].transpose(1, 0, 2).reshape(C, C)
        qa, qb = _strip_pack_cols((ln1_g[l][:, None] * wq * scale).astype(f16))
        ka, kb = _strip_pack_cols((ln1_g[l][:, None] * wk).astype(f16))
        WqTA[l], WqTB[l] = qa, qb
        WkTA[l], WkTB[l] = ka, kb
        WvT[l] = (ln1_g[l][:, None] * wv).astype(f16)
        ba, bb = _strip_pack_rows((ln1_b[l] @ wq) * scale)
        bqA[l], bqB[l] = ba, bb
        ba, bb = _strip_pack_rows(ln1_b[l] @ wk)
        bkA[l], bkB[l] = ba, bb
        bv_tile[l] = np.broadcast_to((ln1_b[l] @ wv)[None, :], (128, C)).astype(f16)

    WpA = np.zeros((Lw, 128, C), f16); WpB = np.zeros((Lw, 128, C), f16)
    for l in range(Lw):
        wp = Wproj[l].reshape(H, D, C)
        for j in range(4):
            WpA[l, 32 * j : 32 * j + 16, :] = wp[j]
            WpB[l, 32 * j : 32 * j + 16, :] = wp[4 + j]
    g["bproj"] = np.ascontiguousarray(bproj.reshape(Lw, C, 1))

    W1T = np.zeros((Lw, C, FF), f16)
    W2T = np.zeros((Lw, 4, C, C), f16)
    for l in range(Lw):
        W1T[l] = (ln2_g[l][:, None] * W1[l]).astype(f16)
        W2T[l] = np.ascontiguousarray(W2[l].reshape(4, 128, C)).astype(f16)
    b1_eff = np.einsum("lc,lcf->lf", ln2_b, W1) + b1
    g["b1"] = np.ascontiguousarray(b1_eff.reshape(Lw, 4, 128, 1)).astype(f32)
    g["b2"] = np.ascontiguousarray(b2.reshape(Lw, C, 1))

    lnf_g = np.asarray(inputs["lnf_g"], f32); lnf_b = np.asarray(inputs["lnf_b"], f32)
    Wlm = np.asarray(inputs["Wlm"], f32); blm = np.asarray(inputs["blm"], f32)
    WlmT = np.zeros((C, VPAD), f32)
    WlmT[:, :VOCAB] = lnf_g[:, None] * Wlm
    g["WlmT"] = WlmT
    g["_host_out_bias"] = (blm + lnf_b @ Wlm).astype(f32)

    g.update(WqTA=WqTA, WqTB=WqTB, WkTA=WkTA, WkTB=WkTB, WvT=WvT,
             bqA=bqA, bqB=bqB, bkA=bkA, bkB=bkB, bv_tile=bv_tile,
             WpA=WpA, WpB=WpB, W1T=W1T, W2T=W2T)

    g["ones_blk_h"] = np.ones((C, 32), f16)
    ones_pad = np.zeros((97, 128), f16)     # bc lhsT at rows {0,32,64,96}
    for j in range(4):
        ones_pad[32 * j, :] = 1.0
    g["ones_pad"] = ones_pad
    G4pad = np.zeros((36, 128), f16)        # rinv bc lhsT at rows 0-3 / 32-35
    for j in range(4):
        G4pad[j, 32 * j : 32 * j + 32] = 1.0
        G4pad[32 + j, 32 * j : 32 * j + 32] = 1.0
    g["G4pad"] = G4pad
    tri = np.zeros((128, 128), f16)         # mask[s, tloc] = 1 if tloc >= s
    for s in range(128):
        tri[s, s:] = 1.0
    g["tri"] = tri
    v16init = np.zeros((128, 2 * BPC * 8 * 32), f16)
    for blk in range(2 * BPC):
        for h in range(8):
            v16init[:, blk * 256 + 32 * h + 16] = 1.0
    g["v16init"] = v16init
    g["ident"] = np.eye(128, dtype=f32)
    g["epsb"] = np.full((128, 1), EPS, f32)
    return g


GLOBAL_SPECS = [
    ("tok_emb", F32), ("posT", F32),
    ("WqTA", F16), ("WqTB", F16), ("WkTA", F16), ("WkTB", F16), ("WvT", F16),
    ("bqA", F32), ("bqB", F32), ("bkA", F32), ("bkB", F32), ("bv_tile", F16),
    ("WpA", F16), ("WpB", F16), ("bproj", F32),
    ("W1T", F16), ("W2T", F16), ("b1", F32), ("b2", F32),
    ("WlmT", F32R),
    ("ones_blk_h", F16), ("ones_pad", F16),
    ("G4pad", F16), ("tri", F16), ("v16init", F16), ("ident", F32),
    ("epsb", F32),
]


def _flatten_w(arr):
    """[..., p, f] -> [p, prod(lead)*f] column-stacked for SBUF residence."""
    if arr.ndim == 2:
        return arr
    lead = int(np.prod(arr.shape[:-2]))
    p, f = arr.shape[-2], arr.shape[-1]
    a = arr.reshape(lead, p, f).transpose(1, 0, 2).reshape(p, lead * f)
    return np.ascontiguousarray(a)


def build(n_layers=NLAYER, loop=1):
    _install_act_table_pin()
    shapes = {k: v.shape for k, v in host_prep_dummy().items() if not k.startswith("_")}
    nc = bacc.Bacc("TRN2", debug=False, num_devices=NCORES)

    dram = {}
    for name, dt in GLOBAL_SPECS:
        shp = shapes[name]
        flat = (int(shp[-2]), int(np.prod(shp[:-2], dtype=int) * shp[-1]))
        dram[name] = nc.dram_tensor(name, flat, dt, kind="ExternalInput").ap()
    tokens = nc.dram_tensor("tokens", (N, 1), U32, kind="ExternalInput").ap()
    logits_out = nc.dram_tensor("logits", (N, VOCAB), F16, kind="ExternalOutput").ap()

    with tile.TileContext(nc) as tc, ExitStack() as ctx:
        wpool = ctx.enter_context(tc.tile_pool(name="w", bufs=1))
        state = ctx.enter_context(tc.tile_pool(name="state", bufs=1))
        pstat = ctx.enter_context(tc.tile_pool(name="pstat", bufs=1, space="PSUM"))
        pbc = ctx.enter_context(tc.tile_pool(name="pbc", bufs=1, space="PSUM"))
        pmm = ctx.enter_context(tc.tile_pool(name="pmm", bufs=2, space="PSUM"))
        psS = ctx.enter_context(tc.tile_pool(name="psS", bufs=1, space="PSUM"))
        psAV = ctx.enter_context(tc.tile_pool(name="psAV", bufs=1, space="PSUM"))
        sbE = ctx.enter_context(tc.tile_pool(name="sbE", bufs=3))

        w = {}
        for name, dt in GLOBAL_SPECS:
            if name == "tok_emb":
                continue
            shape = shapes[name]
            flat = [int(shape[-2]), int(np.prod(shape[:-2], dtype=int) * shape[-1])]
            t_ = wpool.tile(flat, dt, tag=f"w_{name}")
            nc.sync.dma_start(t_[:], dram[name][:])
            w[name] = (t_, shape)

        def W(name, *idx):
            t_, shape = w[name]
            if not idx:
                return t_[:]
            lead = shape[:-2]
            flat_idx = 0
            for dim, i in zip(lead, idx):
                flat_idx = flat_idx * dim + i
            cols = shape[-1]
            return t_[:, flat_idx * cols : (flat_idx + 1) * cols]

        # persistent state
        x_a = state.tile([128, N], F32R, tag="x_a")
        x_b = state.tile([128, N], F32R, tag="x_b")
        xh = state.tile([128, N], F16, tag="xh")
        xsq = state.tile([128, N], F16, tag="xsq")
        t1 = state.tile([128, N], F16, tag="t1")
        xn = state.tile([128, N], F16, tag="xn")
        a16 = state.tile([128, N], F16, tag="a16")
        q16a = state.tile([128, N], F16, tag="q16a")
        q16b = state.tile([128, N], F16, tag="q16b")
        k16a = state.tile([128, N], F16, tag="k16a")
        k16b = state.tile([128, N], F16, tag="k16b")
        qg = [q16a, q16b]
        kg = [k16a, k16b]
        v16 = state.tile([128, 2 * BPC * 256], F16, tag="v16")
        pt16 = state.tile([128, 8 * 384], F16, tag="pt16")
        attn32 = state.tile([128, BPC * 512], F32R, tag="attn32")
        attnn = state.tile([128, BPC * 512], F16, tag="attnn")
        rcoll = state.tile([36, BPC * 256], F32R, tag="rcoll")
        rinv = state.tile([36, BPC * 256], F16, tag="rinv")
        h16 = state.tile([128, 4 * N], F16, tag="h16")
        tf = state.tile([128, N], F32R, tag="tf")
        xnf = state.tile([128, N], F32R, tag="xnf")
        mu_t = state.tile([128, 512], F16, tag="mu_t")
        A_t = state.tile([128, 512], F16, tag="A_t")
        sqmu = state.tile([128, 512], F16, tag="sqmu")
        varp = state.tile([128, 512], F32, tag="varp")
        sd_t = state.tile([128, 512], F32, tag="sd_t")
        idx_sb = state.tile([128, TCH], U32, tag="idx")

        nc.sync.dma_start(v16[:], dram["v16init"][:])
        nc.sync.dma_start(idx_sb[:], tokens.rearrange("(c p) one -> p (c one)", p=128))

        ev_ct = [0]

        def evac_alt(out_ap, in_ap, func=AF.Copy, bias=0.0):
            ev_ct[0] ^= 1
            if ev_ct[0]:
                if func == AF.Copy and isinstance(bias, float) and bias == 0.0:
                    nc.scalar.activation(out_ap, in_ap, AF.Copy)
                else:
                    nc.scalar.activation(out_ap, in_ap, func, bias=bias)
            else:
                if func == AF.Relu:
                    nc.vector.tensor_scalar(out_ap, in_ap, bias, 0.0,
                                            op0=OP.add, op1=OP.max)
                else:
                    nc.vector.tensor_copy(out_ap, in_ap)

        def embed():
            for c in range(TCH):
                gat = sbE.tile([128, 128], F32, tag="gat")
                nc.gpsimd.indirect_dma_start(
                    gat[:], None, dram["tok_emb"][:],
                    bass.IndirectOffsetOnAxis(ap=idx_sb[:, c : c + 1], axis=0),
                )
                tp = pmm.tile([128, 512], F32, tag="mm")
                nc.tensor.transpose(tp[:, 0:128], gat[:], W("ident"))
                nc.vector.tensor_tensor(
                    x_a[:, 128 * c : 128 * (c + 1)], tp[:, 0:128],
                    W("posT")[:, (c % 2) * 128 : (c % 2) * 128 + 128],
                    op=OP.add,
                )

        def layernorm(x_cur, out_xn, final=False):
            nc.vector.tensor_copy(xh[:], x_cur[:])
            nc.vector.tensor_tensor(xsq[:], xh[:], xh[:], op=OP.mult)
            sp1t = pstat.tile([128, 512], F32, tag="statps")
            sp2t = pmm.tile([128, 512], F32, tag="mm")
            for j in range(SCH):
                nc.tensor.matmul(
                    sp1t[32 * j : 32 * j + 32, :], W("ones_blk_h"),
                    xh[:, 512 * j : 512 * (j + 1)],
                    start=True, stop=True, tile_position=(0, 32 * j),
                )
            for j in range(SCH):
                nc.tensor.matmul(
                    sp2t[32 * j : 32 * j + 32, :], W("ones_blk_h"),
                    xsq[:, 512 * j : 512 * (j + 1)],
                    start=True, stop=True, tile_position=(0, 32 * j),
                )
            nc.scalar.activation(mu_t[:], sp1t[:], AF.Copy, scale=1.0 / C)
            nc.vector.tensor_tensor(sqmu[:], mu_t[:], mu_t[:], op=OP.mult)
            nc.vector.scalar_tensor_tensor(
                varp[:], sp2t[:], 1.0 / C, sqmu[:],
                op0=OP.mult, op1=OP.subtract,
            )
            nc.scalar.activation(sd_t[:], varp[:], AF.Ln, bias=W("epsb"))
            nc.scalar.activation(A_t[:], sd_t[:], AF.Exp, scale=-0.5)
            for j in range(SCH):
                bcm = pbc.tile([128, 512], F32, tag="bcmu")
                bca = pbc.tile([128, 512], F32, tag="bcA")
                op_row = W("ones_pad")[32 * j : 32 * j + 1, :]
                nc.tensor.matmul(bcm[:], op_row, mu_t[32 * j : 32 * j + 1, :],
                                 start=True, stop=True, tile_position=(32 * j, 0))
                nc.tensor.matmul(bca[:], op_row, A_t[32 * j : 32 * j + 1, :],
                                 start=True, stop=True, tile_position=(32 * j, 0))
                sl = slice(512 * j, 512 * (j + 1))
                if final:
                    nc.vector.scalar_tensor_tensor(
                        tf[:, sl], x_cur[:, sl], 0.0, bcm[:],
                        op0=OP.add, op1=OP.subtract,
                    )
                    nc.vector.scalar_tensor_tensor(
                        out_xn[:, sl], tf[:, sl], 1.0, bca[:],
                        op0=OP.mult, op1=OP.mult,
                    )
                else:
                    nc.vector.scalar_tensor_tensor(
                        t1[:, sl], x_cur[:, sl], 0.0, bcm[:],
                        op0=OP.add, op1=OP.subtract,
                    )
                    nc.vector.scalar_tensor_tensor(
                        out_xn[:, sl], t1[:, sl], 1.0, bca[:],
                        op0=OP.mult, op1=OP.mult,
                    )

        def layer(l, x_cur, x_next):
            layernorm(x_cur, xn)
            # q, k (striped head groups)
            for j in range(SCH):
                sl = slice(512 * j, 512 * (j + 1))
                for gi, (wq_, bq_, wk_, bk_) in enumerate(
                    [("WqTA", "bqA", "WkTA", "bkA"), ("WqTB", "bqB", "WkTB", "bkB")]
                ):
                    qp = pmm.tile([128, 512], F32, tag="mm")
                    nc.tensor.matmul(qp[:], W(wq_, l), xn[:, sl], start=True, stop=True)
                    nc.scalar.activation(qg[gi][:, sl], qp[:], AF.Identity,
                                         bias=W(bq_, l))
                    kp = pmm.tile([128, 512], F32, tag="mm")
                    nc.tensor.matmul(kp[:], W(wk_, l), xn[:, sl], start=True, stop=True)
                    if gi == 0:
                        nc.vector.tensor_scalar(kg[gi][:, sl], kp[:], W(bk_, l),
                                                0.0, op0=OP.add, op1=OP.add)
                    else:
                        nc.scalar.activation(kg[gi][:, sl], kp[:], AF.Identity,
                                             bias=W(bk_, l))
            # v token-major
            for c in range(TCH):
                vp = pmm.tile([128, 512], F32, tag="mm")
                nc.tensor.matmul(vp[:, 0:128], xn[:, 128 * c : 128 * (c + 1)],
                                 W("WvT", l), start=True, stop=True)
                dst = v16[:].rearrange("p (blk h x) -> p blk h x", blk=2 * BPC, h=8)
                nc.vector.tensor_tensor(
                    dst[:, c, :, 0:16],
                    vp[:, 0:128].rearrange("p (h x) -> p h x", h=8),
                    W("bv_tile", l).rearrange("p (h x) -> p h x", h=8),
                    op=OP.add,
                )
            for b in range(BPC):
                # scores + exp (heads in pairs sharing one psum bank)
                for g in range(2):
                    for jp in range(2):  # strip pairs (0,1) then (2,3)
                        spp = psS.tile([128, 512], F32, tag="sps")
                        sp1 = psS.tile([128, 256], F32, tag="sps1")
                        for j in (2 * jp, 2 * jp + 1):
                            h = 4 * g + j
                            rs = slice(32 * j, 32 * j + 16)
                            co = 256 * (j - 2 * jp)
                            nc.tensor.matmul(
                                spp[:, co : co + 256],
                                kg[g][rs, 256 * b : 256 * b + 128],
                                qg[g][rs, 256 * b : 256 * b + 256],
                                start=True, stop=True, tile_position=(32 * j, 0),
                            )
                            nc.tensor.matmul(
                                sp1[:, co // 2 : co // 2 + 128],
                                kg[g][rs, 256 * b + 128 : 256 * b + 256],
                                qg[g][rs, 256 * b + 128 : 256 * b + 256],
                                start=True, stop=True, tile_position=(32 * j, 0),
                            )
                            nc.scalar.activation(
                                pt16[:, 384 * h : 384 * h + 256],
                                spp[:, co : co + 256], AF.Exp)
                            nc.scalar.activation(
                                pt16[:, 384 * h + 256 : 384 * h + 384],
                                sp1[:, co // 2 : co // 2 + 128], AF.Exp)
                # batched causal mask
                msk = pt16[:].rearrange("p (h k x) -> p h k x", h=8, k=3)[:, :, 0::2, :]
                nc.vector.tensor_tensor(
                    msk, msk,
                    W("tri").unsqueeze(1).unsqueeze(1).broadcast_to([128, 8, 2, 128]),
                    op=OP.mult,
                )
                # AV col-tiled + softmax sums
                for g in range(2):
                    av = psAV.tile([128, 256], F32, tag="avps")
                    for j in range(4):
                        h = 4 * g + j
                        base0 = (2 * b) * 256 + 32 * h
                        base1 = (2 * b + 1) * 256 + 32 * h
                        nc.tensor.matmul(
                            av[32 * j : 32 * j + 32, :],
                            v16[:, base0 : base0 + 32],
                            pt16[:, 384 * h : 384 * h + 256],
                            start=True, stop=False, tile_position=(0, 32 * j),
                        )
                        nc.tensor.matmul(
                            av[32 * j : 32 * j + 32, 128:256],
                            v16[:, base1 : base1 + 32],
                            pt16[:, 384 * h + 256 : 384 * h + 384],
                            start=False, stop=True, tile_position=(0, 32 * j),
                        )
                    nc.vector.tensor_copy(
                        attn32[:, 512 * b + 256 * g : 512 * b + 256 * g + 256], av[:])
                for g in range(2):
                    nc.sync.dma_start(
                        rcoll[32 * g : 32 * g + 4, 256 * b : 256 * b + 256],
                        attn32[16::32, 512 * b + 256 * g : 512 * b + 256 * g + 256],
                    )
                    with nc.allow_low_precision("softmax denom as f16 rhs"):
                        nc.vector.reciprocal(
                            rinv[32 * g : 32 * g + 4, 256 * b : 256 * b + 256],
                            rcoll[32 * g : 32 * g + 4, 256 * b : 256 * b + 256])
                bcr = pbc.tile([128, 512], F32, tag="bcA")
                nc.tensor.matmul(bcr[:, 0:256], W("G4pad")[0:4, :],
                                 rinv[0:4, 256 * b : 256 * b + 256],
                                 start=True, stop=True, tile_position=(0, 0))
                nc.tensor.matmul(bcr[:, 256:512], W("G4pad")[32:36, :],
                                 rinv[32:36, 256 * b : 256 * b + 256],
                                 start=True, stop=True, tile_position=(32, 0))
                nc.vector.scalar_tensor_tensor(
                    attnn[:, 512 * b : 512 * b + 512],
                    attn32[:, 512 * b : 512 * b + 512], 1.0, bcr[:],
                    op0=OP.mult, op1=OP.mult,
                )
            # proj + residual
            att4 = attnn[:].rearrange("p (b g x) -> p b g x", b=BPC, g=2)
            for j in range(SCH):
                pp = pmm.tile([128, 512], F32, tag="mm")
                nc.tensor.matmul(pp[:], W("WpA", l), att4[:, 2 * j : 2 * j + 2, 0, :],
                                 start=True, stop=False)
                nc.tensor.matmul(pp[:], W("WpB", l), att4[:, 2 * j : 2 * j + 2, 1, :],
                                 start=False, stop=True)
                sl = slice(512 * j, 512 * (j + 1))
                nc.vector.scalar_tensor_tensor(
                    x_next[:, sl], pp[:], W("bproj", l), x_cur[:, sl],
                    op0=OP.add, op1=OP.add,
                )
            # FFN
            layernorm(x_next, xn)
            for j in range(SCH):
                sl = slice(512 * j, 512 * (j + 1))
                for f in range(4):
                    hp = pmm.tile([128, 512], F32, tag="mm")
                    nc.tensor.matmul(hp[:], W("W1T", l)[:, 128 * f : 128 * (f + 1)],
                                     xn[:, sl], start=True, stop=True)
                    evac_alt(h16[:, 2048 * f + 512 * j : 2048 * f + 512 * (j + 1)],
                             hp[:], func=AF.Relu, bias=W("b1", l, f))
            for j in range(SCH):
                sl = slice(512 * j, 512 * (j + 1))
                wp2 = pmm.tile([128, 512], F32, tag="mm")
                for f in range(4):
                    nc.tensor.matmul(wp2[:], W("W2T", l, f),
                                     h16[:, 2048 * f + 512 * j : 2048 * f + 512 * (j + 1)],
                                     start=(f == 0), stop=(f == 3))
                nc.vector.scalar_tensor_tensor(
                    x_cur[:, sl], wp2[:], W("b2", l), x_next[:, sl],
                    op0=OP.add, op1=OP.add,
                )

        def lm_head(x_cur):
            layernorm(x_cur, xnf, final=True)
            for c in range(TCH):
                lhsT = xnf[:, 128 * c : 128 * (c + 1)]
                for v in range(VCH):
                    lp = pmm.tile([128, 512], F32, tag="mm")
                    nc.tensor.matmul(lp[:], lhsT,
                                     W("WlmT")[:, 512 * v : 512 * (v + 1)],
                                     start=True, stop=True)
                    cols = min(512, VOCAB - 512 * v)
                    lg = sbE.tile([128, 512], F16, tag="lgt")
                    evac_alt(lg[:, 0:cols], lp[:, 0:cols])
                    nc.sync.dma_start(
                        logits_out[128 * c : 128 * (c + 1), 512 * v : 512 * v + cols],
                        lg[:, 0:cols],
                    )

        for _ in range(loop):
            embed()
            for l in range(n_layers):
                layer(l, x_a, x_b)
            lm_head(x_a)

    nc.compile()
    return nc


_dummy_cache = None


def host_prep_dummy():
    global _dummy_cache
    if _dummy_cache is None:
        dummy = {
            "tok_emb": np.zeros((VOCAB, C), np.float32),
            "pos_emb": np.zeros((T, C), np.float32),
            "Wq": np.zeros((NLAYER, H, C, D), np.float32),
            "Wk": np.zeros((NLAYER, H, C, D), np.float32),
            "Wv": np.zeros((NLAYER, H, C, D), np.float32),
            "Wproj": np.zeros((NLAYER, C, C), np.float32),
            "bproj": np.zeros((NLAYER, C), np.float32),
            "ln1_g": np.ones((NLAYER, C), np.float32),
            "ln1_b": np.zeros((NLAYER, C), np.float32),
            "ln2_g": np.ones((NLAYER, C), np.float32),
            "ln2_b": np.zeros((NLAYER, C), np.float32),
            "W1": np.zeros((NLAYER, C, FF), np.float32),
            "b1": np.zeros((NLAYER, FF), np.float32),
            "W2": np.zeros((NLAYER, FF, C), np.float32),
            "b2": np.zeros((NLAYER, C), np.float32),
            "lnf_g": np.ones((C,), np.float32),
            "lnf_b": np.zeros((C,), np.float32),
            "Wlm": np.zeros((C, VOCAB), np.float32),
            "blm": np.zeros((VOCAB,), np.float32),
        }
        _dummy_cache = host_prep(dummy)
    return _dummy_cache


_nc_cache = {}


def _get_nc(loop=1):
    if loop not in _nc_cache:
        _nc_cache[loop] = build(loop=loop)
    return _nc_cache[loop]


def kernel(**inputs):
    g = host_prep(inputs)
    tokens = np.asarray(inputs["tokens"]).astype(np.uint32)
    shared = {name: _flatten_w(np.ascontiguousarray(g[name]))
              for name, _ in GLOBAL_SPECS}
    in_maps = []
    for core in range(NCORES):
        m = dict(shared)
        tk = tokens[core * BPC : (core + 1) * BPC].reshape(N, 1)
        m["tokens"] = np.ascontiguousarray(tk)
        in_maps.append(m)
    nc = _get_nc(loop=1)
    res = run_bass_kernel_spmd(nc, in_maps, core_ids=list(range(NCORES)))
    out = np.concatenate(
        [res.results[c]["logits"].astype(np.float32).reshape(BPC, T, VOCAB)
         for c in range(NCORES)],
        axis=0,
    )
    hob = g["_host_out_bias"]
    if np.any(hob != 0.0):
        out = out + hob[None, None, :]
    return out.astype(np.float32)


if __name__ == "__main__":
    build(n_layers=1)
    print("built OK")

